# revision 17
# baseline (speedup 1.0000x reference)
"""Trainium2 Bass kernel for nn_LogicLayer (difflogic LogicLayer forward).

Computation (reference):
    w  = softmax(weights, axis=-1)            # [OUT, 16]
    c  = w @ GATE_M                           # [OUT, 4]
    a  = x[:, idx_a]; b = x[:, idx_b]         # [B, OUT] feature gathers
    out = c0 + c1*a + c2*b + c3*(a*b)

Strategy (8 NeuronCores, feature-parallel, division-form math):
  - x uploaded transposed twice: xT8 (u8, q=rint(x*255)) for a-gathers,
    xT16 (bf16) for b-gathers. Each core: 2048 features x full batch,
    16 chunks of 128 features.
  - Division form:  out = (c3*a + c2)*(b + c1/c3) + (c0 - c1*c2/c3).
    With the output code o = SCALE*out + OFF this becomes
        v' = S3*qa + S2          (ScalarE IDENT: free per-partition affine)
        b' = b + alpha           (DVE tensor_scalar add, bf16 4x mode —
                                  or ScalarE IDENT on some chunks, balance)
        m  = v' * b'             (DVE tensor_tensor, bf16 2x)
        o8 = m + beta' -> u8     (DVE tensor_scalar add, u8 out, 2x_2p)
    All three non-TT ops are per-partition-scalar affines, so the only
    expensive TT is the multiply, and the final pass emits u8 directly
    (plain HWDGE store, half the output bytes, no cast DMA).
  - Features where alpha = c1/c3 is ill-conditioned (|c3| small) are
    host-permuted into ONE "unsafe" chunk per core (chunk 15) computed
    with the classic form u + v*b (final TT at 1x, only 1 chunk pays).
    Host inverse-permutes the output columns.
  - Gate coefficients computed on-device from `weights` (exp on ScalarE,
    strided-AP reductions on VectorE), as in the reference softmax path.
"""

import numpy as np

BATCH, IN_DIM, OUT_DIM = 4096, 16384, 16384
N_CORES = 8
F_CORE = OUT_DIM // N_CORES  # 2048 output features per core
P = 128
N_CHUNKS = F_CORE // P  # 16
UNSAFE_CI = 7  # chunk holding the ill-conditioned features (mid-stream)

SCALE = 248.0  # out -> u8 code scale
OFF = 3.5  # keeps codes in [~1.5, ~253.5]: safe from wrap/saturate
CAST_GAMMA = 3.5  # host-side un-bias (3.0 if conversion truncates)

_GATE_M = np.array(
    [
        [0.0, 0.0, 0.0, 0.0], [0.0, 0.0, 0.0, 1.0],
        [0.0, 1.0, 0.0, -1.0], [0.0, 1.0, 0.0, 0.0],
        [0.0, 0.0, 1.0, -1.0], [0.0, 0.0, 1.0, 0.0],
        [0.0, 1.0, 1.0, -2.0], [0.0, 1.0, 1.0, -1.0],
        [1.0, -1.0, -1.0, 1.0], [1.0, -1.0, -1.0, 2.0],
        [1.0, 0.0, -1.0, 0.0], [1.0, 0.0, -1.0, 1.0],
        [1.0, -1.0, 0.0, 0.0], [1.0, -1.0, 0.0, 1.0],
        [1.0, 0.0, 0.0, -1.0], [1.0, 0.0, 0.0, 0.0],
    ],
    dtype=np.float64,
)


def _build_nc(in_dim, feat_core, batch):
    """Build + compile the per-core Bass program (SPMD, identical cores)."""
    from contextlib import ExitStack

    import concourse.bacc as bacc
    import concourse.mybir as mybir
    import concourse.tile as tile

    F32 = mybir.dt.float32
    BF16 = mybir.dt.bfloat16
    U8 = mybir.dt.uint8
    I16 = mybir.dt.int16
    TT = feat_core // P  # feature chunks per core (16)
    mult = mybir.AluOpType.mult
    add = mybir.AluOpType.add
    subtract = mybir.AluOpType.subtract
    Ident = mybir.ActivationFunctionType.Identity

    nc = bacc.Bacc(
        "TRN2", target_bir_lowering=False, debug=False, num_swdge_queues=2
    )
    xT8 = nc.dram_tensor("xT8", [in_dim, batch], U8, kind="ExternalInput")
    xT16 = nc.dram_tensor("xT16", [in_dim, batch], BF16, kind="ExternalInput")
    w = nc.dram_tensor("w", [feat_core, 16], F32, kind="ExternalInput")
    # combined gather indices: per chunk, 128 idx_a then 128 idx_b
    idx = nc.dram_tensor("idx", [P, 2 * feat_core // 16], I16, kind="ExternalInput")
    outT8 = nc.dram_tensor("outT8", [feat_core, batch], U8, kind="ExternalOutput")

    with tile.TileContext(nc) as tc, ExitStack() as ctx:
        const_pool = ctx.enter_context(tc.tile_pool(name="const", bufs=1))
        g_pool = ctx.enter_context(tc.tile_pool(name="g", bufs=3))
        uv_pool = ctx.enter_context(tc.tile_pool(name="uv", bufs=4))

        # Warmup: a tiny gather on a zeroed index tile, issued before any
        # data arrives, pays the ~6us one-time Q7 ext-isa IRAM load while
        # the idx/weights DMAs and coefficient math run.
        wu_idx = const_pool.tile([P, 1], I16, tag="wuidx")
        nc.gpsimd.memset(wu_idx[:], 0)
        wu_out = const_pool.tile([P, 1, batch], U8, tag="wuout")
        nc.gpsimd.dma_gather(
            wu_out[:], xT8[:], wu_idx[:], 16, 16, batch, queue_num=0
        )

        # pair-0 indices in their own tiny tile so the first gather only
        # waits on a 64 B/partition DMA, not the full index load
        idx0_sb = const_pool.tile([P, 32], I16, tag="idx0")
        nc.sync.dma_start(idx0_sb[:], idx[:, 0:32])
        idx_sb = const_pool.tile([P, 2 * feat_core // 16], I16, tag="idx")
        nc.sync.dma_start(idx_sb[:, 32:], idx[:, 32:])

        sc2 = const_pool.tile([P, TT], F32, tag="sc2")  # SCALE*c2
        sc3 = const_pool.tile([P, TT], F32, tag="sc3")  # SCALE*c3/255
        alp = const_pool.tile([P, TT], F32, tag="alp")  # c1/c3
        bet = const_pool.tile([P, TT], F32, tag="bet")  # SCALE*(c0-c1c2/c3)+OFF
        uc0 = const_pool.tile([P, TT], F32, tag="uc0")  # SCALE*c0 + OFF
        uc1 = const_pool.tile([P, TT], F32, tag="uc1")  # SCALE*c1/255

        # ---------- gate coefficients ----------
        # Setup pool stays open for the kernel's lifetime (~5 KB/partition):
        # closing it would put a scope-exit barrier in front of the first
        # gather (~8 us of serialized lead-in).
        sp = ctx.enter_context(tc.tile_pool(name="setup", bufs=1))
        if True:
            w_sb = sp.tile([P, TT, 16], F32, tag="wsb")
            nc.sync.dma_start(w_sb[:], w[:].rearrange("(t p) g -> p t g", p=P))
            E = sp.tile([P, TT, 16], F32, tag="E")
            nc.scalar.activation(E[:], w_sb[:], mybir.ActivationFunctionType.Exp)

            su = sp.tile([P, TT], F32, tag="su")
            nc.vector.reduce_sum(su[:], E[:], axis=mybir.AxisListType.X)
            r = sp.tile([P, TT], F32, tag="r")
            nc.vector.reciprocal(r[:], su[:])

            c0u = sp.tile([P, TT], F32, tag="c0u")
            nc.vector.reduce_sum(c0u[:], E[:, :, 8:16], axis=mybir.AxisListType.X)

            E4 = E[:].rearrange("p t (g2 g1) -> p t g2 g1", g1=4)
            a1 = sp.tile([P, TT], F32, tag="a1")
            nc.vector.reduce_sum(a1[:], E4[:, :, 0:2, 2:4], axis=mybir.AxisListType.XY)
            b1 = sp.tile([P, TT], F32, tag="b1")
            nc.vector.reduce_sum(b1[:], E4[:, :, 2:4, 0:2], axis=mybir.AxisListType.XY)
            c1u = sp.tile([P, TT], F32, tag="c1u")
            nc.vector.tensor_tensor(c1u[:], a1[:], b1[:], op=subtract)

            a2 = sp.tile([P, TT], F32, tag="a2")
            nc.vector.reduce_sum(a2[:], E[:, :, 4:8], axis=mybir.AxisListType.X)
            b2 = sp.tile([P, TT], F32, tag="b2")
            nc.vector.reduce_sum(b2[:], E[:, :, 8:12], axis=mybir.AxisListType.X)
            c2u = sp.tile([P, TT], F32, tag="c2u")
            nc.vector.tensor_tensor(c2u[:], a2[:], b2[:], op=subtract)

            # c3 = (E1+E8) + (E11+E13) - (E2+E4) - (E7+E14) - 2*(E6-E9)
            def eg(g):
                return E[:, :, g : g + 1]

            p1 = sp.tile([P, TT, 1], F32, tag="p1")
            nc.vector.tensor_tensor(p1[:], eg(1), eg(8), op=add)
            p2 = sp.tile([P, TT, 1], F32, tag="p2")
            nc.vector.tensor_tensor(p2[:], eg(11), eg(13), op=add)
            n1 = sp.tile([P, TT, 1], F32, tag="n1")
            nc.vector.tensor_tensor(n1[:], eg(2), eg(4), op=add)
            n2 = sp.tile([P, TT, 1], F32, tag="n2")
            nc.vector.tensor_tensor(n2[:], eg(7), eg(14), op=add)
            d6 = sp.tile([P, TT, 1], F32, tag="d6")
            nc.vector.tensor_tensor(d6[:], eg(6), eg(9), op=subtract)
            pp = sp.tile([P, TT, 1], F32, tag="pp")
            nc.vector.tensor_tensor(pp[:], p1[:], p2[:], op=add)
            nn_ = sp.tile([P, TT, 1], F32, tag="nn")
            nc.vector.tensor_tensor(nn_[:], n1[:], n2[:], op=add)
            c3a = sp.tile([P, TT, 1], F32, tag="c3a")
            nc.vector.tensor_tensor(c3a[:], pp[:], nn_[:], op=subtract)
            c3u = sp.tile([P, TT, 1], F32, tag="c3u")
            nc.vector.scalar_tensor_tensor(
                c3u[:], d6[:], -2.0, c3a[:], op0=mult, op1=add
            )
            c3f = c3u[:, :, 0]

            # Folded coefficients. r = 1/sum(E) (softmax norm), qa = 255*a.
            #   v' = sc3*qa + sc2 = SCALE*(c3*a + c2)
            #   o  = v'*(b + alp) + bet ; alp = c1/c3 (r cancels)
            #   bet = SCALE*r*(c0u - alp*c2u) + OFF
            rS = sp.tile([P, TT], F32, tag="rS")
            nc.vector.tensor_scalar_mul(rS[:], r[:], SCALE)
            rS255 = sp.tile([P, TT], F32, tag="rS255")
            nc.vector.tensor_scalar_mul(rS255[:], r[:], SCALE / 255.0)
            nc.vector.tensor_tensor(sc2[:], c2u[:], rS[:], op=mult)
            nc.vector.tensor_tensor(sc3[:], c3f, rS255[:], op=mult)

            rc3 = sp.tile([P, TT], F32, tag="rc3")
            nc.vector.reciprocal(rc3[:], c3f)
            nc.vector.tensor_tensor(alp[:], c1u[:], rc3[:], op=mult)
            t1 = sp.tile([P, TT], F32, tag="t1")
            nc.vector.tensor_tensor(t1[:], alp[:], c2u[:], op=mult)
            t2 = sp.tile([P, TT], F32, tag="t2")
            nc.vector.tensor_tensor(t2[:], c0u[:], t1[:], op=subtract)
            t3 = sp.tile([P, TT], F32, tag="t3")
            nc.vector.tensor_tensor(t3[:], t2[:], rS[:], op=mult)
            nc.vector.tensor_scalar_add(bet[:], t3[:], OFF)

            # classic-form coefficients for the unsafe chunk
            uc0a = sp.tile([P, TT], F32, tag="uc0a")
            nc.vector.tensor_tensor(uc0a[:], c0u[:], rS[:], op=mult)
            nc.vector.tensor_scalar_add(uc0[:], uc0a[:], OFF)
            nc.vector.tensor_tensor(uc1[:], c1u[:], rS255[:], op=mult)

        # ---------- main gather + FMA loop ----------
        # Gathers are issued per chunk-PAIR (256 indices each) to halve the
        # SWDGE descriptor-gen op count on the Q7s (which 2-port DVE ops
        # lock out of SBUF).
        #
        # Emission order is software-pipelined: engine queues are strict
        # FIFO, so per pair we emit gathers, then BOTH v' passes (a-fed),
        # then both b' passes (b-fed), and the DVE finals (m, o8) of the
        # PREVIOUS pair. This keeps every queue's head runnable — no pass
        # sits behind one that waits on a later DMA or another engine.
        o_pool = ctx.enter_context(tc.tile_pool(name="o", bufs=4))
        B_ON_ACT = {1, 3, 5, 9, 11, 13}  # b-shift on ScalarE (balance)

        state = {}  # ci -> (v_tile, b_operand, u_tile or None)

        def emit_front(cj, a_t, b_t):
            for sub in range(2):
                ci = 2 * cj + sub
                cs = slice(ci, ci + 1)
                a_v = a_t[:, sub, :]
                # v' = sc3*qa + sc2  (ScalarE, free per-partition affine)
                v = uv_pool.tile([P, batch], BF16, tag="v")
                nc.scalar.activation(
                    v[:], a_v, Ident, bias=sc2[:, cs], scale=sc3[:, cs]
                )
                state[ci] = [v, None, None]
                if ci == UNSAFE_CI:
                    u = uv_pool.tile([P, batch], BF16, tag="bp")
                    nc.scalar.activation(
                        u[:], a_v, Ident, bias=uc0[:, cs], scale=uc1[:, cs]
                    )
                    state[ci][1] = b_t[:, sub, :]
                    state[ci][2] = u
            for sub in range(2):
                ci = 2 * cj + sub
                cs = slice(ci, ci + 1)
                if ci == UNSAFE_CI:
                    continue
                b_v = b_t[:, sub, :]
                bp = uv_pool.tile([P, batch], BF16, tag="bp")
                if ci in B_ON_ACT:
                    nc.scalar.activation(bp[:], b_v, Ident, bias=alp[:, cs])
                else:
                    nc.vector.tensor_scalar(bp[:], b_v, alp[:, cs], None, add)
                state[ci][1] = bp[:]

        def emit_back(cj):
            for sub in range(2):
                ci = 2 * cj + sub
                cs = slice(ci, ci + 1)
                v, b_op, u = state.pop(ci)
                o8 = o_pool.tile([P, batch], U8, tag="o8")
                nc.vector.tensor_tensor(v[:], v[:], b_op, op=mult)
                if u is None:
                    nc.vector.tensor_scalar(o8[:], v[:], bet[:, cs], None, add)
                else:
                    nc.vector.tensor_tensor(o8[:], v[:], u[:], op=add)
                nc.sync.dma_start(outT8[ci * P : (ci + 1) * P, :], o8[:])

        for cj in range(TT // 2):
            isrc = idx0_sb if cj == 0 else idx_sb
            a_t = g_pool.tile([P, 2, batch], U8, tag="ga")
            nc.gpsimd.dma_gather(
                a_t[:], xT8[:], isrc[:, cj * 32 : cj * 32 + 16], 256, 256, batch,
                queue_num=cj % 2,
            )
            b_t = g_pool.tile([P, 2, batch], BF16, tag="gb16")
            nc.gpsimd.dma_gather(
                b_t[:], xT16[:], isrc[:, cj * 32 + 16 : cj * 32 + 32], 256, 256,
                batch, queue_num=(cj + 1) % 2,
            )
            emit_front(cj, a_t, b_t)
            if cj > 0:
                emit_back(cj - 1)
        emit_back(TT // 2 - 1)

    nc.compile()
    return nc


def _pack_idx(idx_a, idx_b):
    """Host-side int16 gather-index buffer for one core.

    Per 256-feature chunk-PAIR: 256 idx_a then 256 idx_b (one dma_gather
    each). dma_gather consumes index i from partition i%16, column i//16
    (replicated across the 8 groups of 16 partitions).
    """
    cols = []
    for f0 in range(0, len(idx_a), 2 * P):
        ids = np.concatenate(
            [idx_a[f0 : f0 + 2 * P], idx_b[f0 : f0 + 2 * P]]
        ).astype(np.int16)
        blk = ids.reshape(32, 16)  # [col, partition-within-16]
        cols.append(np.tile(blk.T, (P // 16, 1)))  # [128, 32]
    return np.ascontiguousarray(np.concatenate(cols, axis=1))


def _core_perm(weights, lo, hi):
    """Order this core's features: well-conditioned first, the 128 worst
    (by the bf16-magnitude metric of the division form) into the last
    chunk. Returns global feature indices in on-device order."""
    w = weights[lo:hi].astype(np.float64)
    e = np.exp(w - w.max(axis=1, keepdims=True))
    sm = e / e.sum(axis=1, keepdims=True)
    c = sm @ _GATE_M
    c0, c1, c2, c3 = c.T
    with np.errstate(divide="ignore", invalid="ignore"):
        al = c1 / c3
        vs = np.stack([SCALE * c2, SCALE * (c3 + c2)])  # v' at a in {0,1}
        bs = np.stack([al, 1.0 + al])  # b' at b in {0,1}
        m_max = np.max(np.abs(vs[:, None, :] * bs[None, :, :]), axis=(0, 1))
    metric = np.where(np.isfinite(m_max), m_max, np.inf)
    order = np.argsort(metric, kind="stable")
    safe = np.sort(order[: hi - lo - P])
    unsafe = np.sort(order[hi - lo - P :])
    # worst 128 features land in chunk UNSAFE_CI; safe ones fill the rest
    return lo + np.concatenate(
        [safe[: UNSAFE_CI * P], unsafe, safe[UNSAFE_CI * P :]]
    )


_NC_CACHE = {}


def _get_nc():
    key = (IN_DIM, F_CORE, BATCH)
    if key not in _NC_CACHE:
        _NC_CACHE[key] = _build_nc(IN_DIM, F_CORE, BATCH)
    return _NC_CACHE[key]


TRACE = False  # set by dev harness to capture an NTFF profile
LAST_RESULT = None


def kernel(x, weights, idx_a, idx_b):
    global LAST_RESULT
    import ml_dtypes
    from concourse.bass_utils import run_bass_kernel_spmd

    x = np.asarray(x, dtype=np.float32)
    weights = np.asarray(weights, dtype=np.float32)
    idx_a = np.asarray(idx_a)
    idx_b = np.asarray(idx_b)

    nc = _get_nc()
    xT8 = np.ascontiguousarray(np.rint(x * 255.0).astype(np.uint8).T)
    xT16 = np.ascontiguousarray(x.astype(ml_dtypes.bfloat16).T)
    in_maps = []
    perms = []
    for k in range(N_CORES):
        lo, hi = k * F_CORE, (k + 1) * F_CORE
        perm = _core_perm(weights, lo, hi)
        perms.append(perm)
        in_maps.append(
            {
                "xT8": xT8,
                "xT16": xT16,
                "w": np.ascontiguousarray(weights[perm]),
                "idx": _pack_idx(idx_a[perm], idx_b[perm]),
            }
        )

    res = run_bass_kernel_spmd(nc, in_maps, list(range(N_CORES)), trace=TRACE)
    LAST_RESULT = res
    out = np.empty((BATCH, OUT_DIM), dtype=np.float32)
    for k in range(N_CORES):
        q = res.results[k]["outT8"].astype(np.float32)
        out[:, perms[k]] = ((q - CAST_GAMMA) / SCALE).T
    return out


# revision 21
# speedup vs baseline: 1.0599x; 1.0599x over previous
"""Trainium2 Bass kernel for nn_LogicLayer (difflogic LogicLayer forward).

Computation (reference):
    w  = softmax(weights, axis=-1)            # [OUT, 16]
    c  = w @ GATE_M                           # [OUT, 4]
    a  = x[:, idx_a]; b = x[:, idx_b]         # [B, OUT] feature gathers
    out = c0 + c1*a + c2*b + c3*(a*b)

Strategy (8 NeuronCores, feature-parallel, division-form math):
  - x uploaded transposed twice: xT8 (u8, q=rint(x*255)) for a-gathers,
    xT16 (bf16) for b-gathers. Each core: 2048 features x full batch,
    16 chunks of 128 features.
  - Division form:  out = (c3*a + c2)*(b + c1/c3) + (c0 - c1*c2/c3).
    With the output code o = SCALE*out + OFF this becomes
        v' = S3*qa + S2          (ScalarE IDENT: free per-partition affine)
        b' = b + alpha           (DVE tensor_scalar add, bf16 4x mode —
                                  or ScalarE IDENT on some chunks, balance)
        m  = v' * b'             (DVE tensor_tensor, bf16 2x)
        o8 = m + beta' -> u8     (DVE tensor_scalar add, u8 out, 2x_2p)
    All three non-TT ops are per-partition-scalar affines, so the only
    expensive TT is the multiply, and the final pass emits u8 directly
    (plain HWDGE store, half the output bytes, no cast DMA).
  - Features where alpha = c1/c3 is ill-conditioned (|c3| small) are
    host-permuted into ONE "unsafe" chunk per core (chunk 15) computed
    with the classic form u + v*b (final TT at 1x, only 1 chunk pays).
    Host inverse-permutes the output columns.
  - Gate coefficients computed on-device from `weights` (exp on ScalarE,
    strided-AP reductions on VectorE), as in the reference softmax path.
"""

import numpy as np

BATCH, IN_DIM, OUT_DIM = 4096, 16384, 16384
N_CORES = 8
F_CORE = OUT_DIM // N_CORES  # 2048 output features per core
P = 128
N_CHUNKS = F_CORE // P  # 16

# Build-time tuning knobs (bake-off parameterization):
#   unsafe_ci: chunk index holding ill-conditioned features
#   b_on_act:  chunks whose b-shift runs on ScalarE (engine balance)
#   dummy_exp: issue a 1-column Exp first so the ~2.7us ACT table load
#              overlaps the input DMAs instead of the coefficient chain
CONFIG = {
    # unsafe chunk first: its DVE ops are all TTs, which do not contend
    # with the Q7s' shared SBUF port during the gather descriptor burst
    "unsafe_ci": 0,
    "b_on_act": (1, 3, 5, 9, 11, 13),
    "dummy_exp": True,
    # chunks whose output is written as bf16 codes instead of u8: their
    # final TS runs at 4x (1.2us vs 2.35us) at the cost of +0.5 MB HBM
    "bf16_out_chunks": (),
}

SCALE = 248.0  # out -> u8 code scale
OFF = 3.5  # keeps codes in [~1.5, ~253.5]: safe from wrap/saturate
CAST_GAMMA = 3.5  # host-side un-bias (3.0 if conversion truncates)

_GATE_M = np.array(
    [
        [0.0, 0.0, 0.0, 0.0], [0.0, 0.0, 0.0, 1.0],
        [0.0, 1.0, 0.0, -1.0], [0.0, 1.0, 0.0, 0.0],
        [0.0, 0.0, 1.0, -1.0], [0.0, 0.0, 1.0, 0.0],
        [0.0, 1.0, 1.0, -2.0], [0.0, 1.0, 1.0, -1.0],
        [1.0, -1.0, -1.0, 1.0], [1.0, -1.0, -1.0, 2.0],
        [1.0, 0.0, -1.0, 0.0], [1.0, 0.0, -1.0, 1.0],
        [1.0, -1.0, 0.0, 0.0], [1.0, -1.0, 0.0, 1.0],
        [1.0, 0.0, 0.0, -1.0], [1.0, 0.0, 0.0, 0.0],
    ],
    dtype=np.float64,
)


def _build_nc(in_dim, feat_core, batch, cfg):
    """Build + compile the per-core Bass program (SPMD, identical cores)."""
    UNSAFE_CI = cfg["unsafe_ci"]
    from contextlib import ExitStack

    import concourse.bacc as bacc
    import concourse.mybir as mybir
    import concourse.tile as tile

    F32 = mybir.dt.float32
    BF16 = mybir.dt.bfloat16
    U8 = mybir.dt.uint8
    I16 = mybir.dt.int16
    TT = feat_core // P  # feature chunks per core (16)
    mult = mybir.AluOpType.mult
    add = mybir.AluOpType.add
    subtract = mybir.AluOpType.subtract
    Ident = mybir.ActivationFunctionType.Identity

    nc = bacc.Bacc(
        "TRN2", target_bir_lowering=False, debug=False, num_swdge_queues=2
    )
    xT8 = nc.dram_tensor("xT8", [in_dim, batch], U8, kind="ExternalInput")
    xT16 = nc.dram_tensor("xT16", [in_dim, batch], BF16, kind="ExternalInput")
    w = nc.dram_tensor("w", [feat_core, 16], F32, kind="ExternalInput")
    # combined gather indices: per chunk, 128 idx_a then 128 idx_b
    idx = nc.dram_tensor("idx", [P, 2 * feat_core // 16], I16, kind="ExternalInput")
    outT8 = nc.dram_tensor("outT8", [feat_core, batch], U8, kind="ExternalOutput")
    n16 = len(cfg["bf16_out_chunks"])
    outT16 = (
        nc.dram_tensor("outT16", [n16 * P, batch], BF16, kind="ExternalOutput")
        if n16
        else None
    )

    with tile.TileContext(nc) as tc, ExitStack() as ctx:
        const_pool = ctx.enter_context(tc.tile_pool(name="const", bufs=1))
        g_pool = ctx.enter_context(tc.tile_pool(name="g", bufs=3))
        uv_pool = ctx.enter_context(tc.tile_pool(name="uv", bufs=4))

        # Warmup: a tiny gather on a zeroed index tile, issued before any
        # data arrives, pays the ~6us one-time Q7 ext-isa IRAM load while
        # the idx/weights DMAs and coefficient math run.
        wu_idx = const_pool.tile([P, 1], I16, tag="wuidx")
        nc.gpsimd.memset(wu_idx[:], 0)
        wu_out = const_pool.tile([P, 1, batch], U8, tag="wuout")
        nc.gpsimd.dma_gather(
            wu_out[:], xT8[:], wu_idx[:], 16, 16, batch, queue_num=0
        )

        if cfg["dummy_exp"]:
            # prefetch the exp table set while input DMAs run
            dxp = const_pool.tile([P, 1], F32, tag="dxp")
            nc.vector.memset(dxp[:], 0.0)
            nc.scalar.activation(
                dxp[:], dxp[:], mybir.ActivationFunctionType.Exp
            )

        # pair-0 indices in their own tiny tile so the first gather only
        # waits on a 64 B/partition DMA, not the full index load
        idx0_sb = const_pool.tile([P, 32], I16, tag="idx0")
        nc.sync.dma_start(idx0_sb[:], idx[:, 0:32])
        idx_sb = const_pool.tile([P, 2 * feat_core // 16], I16, tag="idx")
        nc.sync.dma_start(idx_sb[:, 32:], idx[:, 32:])

        sc2 = const_pool.tile([P, TT], F32, tag="sc2")  # SCALE*c2
        sc3 = const_pool.tile([P, TT], F32, tag="sc3")  # SCALE*c3/255
        alp = const_pool.tile([P, TT], F32, tag="alp")  # c1/c3
        bet = const_pool.tile([P, TT], F32, tag="bet")  # SCALE*(c0-c1c2/c3)+OFF
        uc0 = const_pool.tile([P, TT], F32, tag="uc0")  # SCALE*c0 + OFF
        uc1 = const_pool.tile([P, TT], F32, tag="uc1")  # SCALE*c1/255

        # ---------- gate coefficients ----------
        # Setup pool stays open for the kernel's lifetime (~5 KB/partition):
        # closing it would put a scope-exit barrier in front of the first
        # gather (~8 us of serialized lead-in).
        sp = ctx.enter_context(tc.tile_pool(name="setup", bufs=1))
        if True:
            w_sb = sp.tile([P, TT, 16], F32, tag="wsb")
            nc.sync.dma_start(w_sb[:], w[:].rearrange("(t p) g -> p t g", p=P))
            E = sp.tile([P, TT, 16], F32, tag="E")
            nc.scalar.activation(E[:], w_sb[:], mybir.ActivationFunctionType.Exp)

            su = sp.tile([P, TT], F32, tag="su")
            nc.vector.reduce_sum(su[:], E[:], axis=mybir.AxisListType.X)
            r = sp.tile([P, TT], F32, tag="r")
            nc.vector.reciprocal(r[:], su[:])

            c0u = sp.tile([P, TT], F32, tag="c0u")
            nc.vector.reduce_sum(c0u[:], E[:, :, 8:16], axis=mybir.AxisListType.X)

            E4 = E[:].rearrange("p t (g2 g1) -> p t g2 g1", g1=4)
            a1 = sp.tile([P, TT], F32, tag="a1")
            nc.vector.reduce_sum(a1[:], E4[:, :, 0:2, 2:4], axis=mybir.AxisListType.XY)
            b1 = sp.tile([P, TT], F32, tag="b1")
            nc.vector.reduce_sum(b1[:], E4[:, :, 2:4, 0:2], axis=mybir.AxisListType.XY)
            c1u = sp.tile([P, TT], F32, tag="c1u")
            nc.vector.tensor_tensor(c1u[:], a1[:], b1[:], op=subtract)

            a2 = sp.tile([P, TT], F32, tag="a2")
            nc.vector.reduce_sum(a2[:], E[:, :, 4:8], axis=mybir.AxisListType.X)
            b2 = sp.tile([P, TT], F32, tag="b2")
            nc.vector.reduce_sum(b2[:], E[:, :, 8:12], axis=mybir.AxisListType.X)
            c2u = sp.tile([P, TT], F32, tag="c2u")
            nc.vector.tensor_tensor(c2u[:], a2[:], b2[:], op=subtract)

            # c3 = (E1+E8) + (E11+E13) - (E2+E4) - (E7+E14) - 2*(E6-E9)
            def eg(g):
                return E[:, :, g : g + 1]

            p1 = sp.tile([P, TT, 1], F32, tag="p1")
            nc.vector.tensor_tensor(p1[:], eg(1), eg(8), op=add)
            p2 = sp.tile([P, TT, 1], F32, tag="p2")
            nc.vector.tensor_tensor(p2[:], eg(11), eg(13), op=add)
            n1 = sp.tile([P, TT, 1], F32, tag="n1")
            nc.vector.tensor_tensor(n1[:], eg(2), eg(4), op=add)
            n2 = sp.tile([P, TT, 1], F32, tag="n2")
            nc.vector.tensor_tensor(n2[:], eg(7), eg(14), op=add)
            d6 = sp.tile([P, TT, 1], F32, tag="d6")
            nc.vector.tensor_tensor(d6[:], eg(6), eg(9), op=subtract)
            pp = sp.tile([P, TT, 1], F32, tag="pp")
            nc.vector.tensor_tensor(pp[:], p1[:], p2[:], op=add)
            nn_ = sp.tile([P, TT, 1], F32, tag="nn")
            nc.vector.tensor_tensor(nn_[:], n1[:], n2[:], op=add)
            c3a = sp.tile([P, TT, 1], F32, tag="c3a")
            nc.vector.tensor_tensor(c3a[:], pp[:], nn_[:], op=subtract)
            c3u = sp.tile([P, TT, 1], F32, tag="c3u")
            nc.vector.scalar_tensor_tensor(
                c3u[:], d6[:], -2.0, c3a[:], op0=mult, op1=add
            )
            c3f = c3u[:, :, 0]

            # Folded coefficients. r = 1/sum(E) (softmax norm), qa = 255*a.
            #   v' = sc3*qa + sc2 = SCALE*(c3*a + c2)
            #   o  = v'*(b + alp) + bet ; alp = c1/c3 (r cancels)
            #   bet = SCALE*r*(c0u - alp*c2u) + OFF
            rS = sp.tile([P, TT], F32, tag="rS")
            nc.vector.tensor_scalar_mul(rS[:], r[:], SCALE)
            rS255 = sp.tile([P, TT], F32, tag="rS255")
            nc.vector.tensor_scalar_mul(rS255[:], r[:], SCALE / 255.0)
            nc.vector.tensor_tensor(sc2[:], c2u[:], rS[:], op=mult)
            nc.vector.tensor_tensor(sc3[:], c3f, rS255[:], op=mult)

            rc3 = sp.tile([P, TT], F32, tag="rc3")
            nc.vector.reciprocal(rc3[:], c3f)
            nc.vector.tensor_tensor(alp[:], c1u[:], rc3[:], op=mult)
            t1 = sp.tile([P, TT], F32, tag="t1")
            nc.vector.tensor_tensor(t1[:], alp[:], c2u[:], op=mult)
            t2 = sp.tile([P, TT], F32, tag="t2")
            nc.vector.tensor_tensor(t2[:], c0u[:], t1[:], op=subtract)
            t3 = sp.tile([P, TT], F32, tag="t3")
            nc.vector.tensor_tensor(t3[:], t2[:], rS[:], op=mult)
            nc.vector.tensor_scalar_add(bet[:], t3[:], OFF)

            # classic-form coefficients for the unsafe chunk
            uc0a = sp.tile([P, TT], F32, tag="uc0a")
            nc.vector.tensor_tensor(uc0a[:], c0u[:], rS[:], op=mult)
            nc.vector.tensor_scalar_add(uc0[:], uc0a[:], OFF)
            nc.vector.tensor_tensor(uc1[:], c1u[:], rS255[:], op=mult)

        # ---------- main gather + FMA loop ----------
        # Gathers are issued per chunk-PAIR (256 indices each) to halve the
        # SWDGE descriptor-gen op count on the Q7s (which 2-port DVE ops
        # lock out of SBUF).
        #
        # Emission order is software-pipelined: engine queues are strict
        # FIFO, so per pair we emit gathers, then BOTH v' passes (a-fed),
        # then both b' passes (b-fed), and the DVE finals (m, o8) of the
        # PREVIOUS pair. This keeps every queue's head runnable — no pass
        # sits behind one that waits on a later DMA or another engine.
        o_pool = ctx.enter_context(tc.tile_pool(name="o", bufs=4))
        B_ON_ACT = set(cfg["b_on_act"])  # b-shift on ScalarE (balance)

        state = {}  # ci -> (v_tile, b_operand, u_tile or None)

        def emit_front(cj, a_t, b_t):
            for sub in range(2):
                ci = 2 * cj + sub
                cs = slice(ci, ci + 1)
                a_v = a_t[:, sub, :]
                # v' = sc3*qa + sc2  (ScalarE, free per-partition affine)
                v = uv_pool.tile([P, batch], BF16, tag="v")
                nc.scalar.activation(
                    v[:], a_v, Ident, bias=sc2[:, cs], scale=sc3[:, cs]
                )
                state[ci] = [v, None, None]
                if ci == UNSAFE_CI:
                    u = uv_pool.tile([P, batch], BF16, tag="bp")
                    nc.scalar.activation(
                        u[:], a_v, Ident, bias=uc0[:, cs], scale=uc1[:, cs]
                    )
                    state[ci][1] = b_t[:, sub, :]
                    state[ci][2] = u
            for sub in range(2):
                ci = 2 * cj + sub
                cs = slice(ci, ci + 1)
                if ci == UNSAFE_CI:
                    continue
                b_v = b_t[:, sub, :]
                bp = uv_pool.tile([P, batch], BF16, tag="bp")
                if ci in B_ON_ACT:
                    nc.scalar.activation(bp[:], b_v, Ident, bias=alp[:, cs])
                else:
                    nc.vector.tensor_scalar(bp[:], b_v, alp[:, cs], None, add)
                state[ci][1] = bp[:]

        bf16_out = list(cfg["bf16_out_chunks"])

        def emit_back(cj):
            for sub in range(2):
                ci = 2 * cj + sub
                cs = slice(ci, ci + 1)
                v, b_op, u = state.pop(ci)
                wide = ci in bf16_out
                o8 = o_pool.tile([P, batch], BF16 if wide else U8,
                                 tag="o16" if wide else "o8")
                nc.vector.tensor_tensor(v[:], v[:], b_op, op=mult)
                if u is None:
                    nc.vector.tensor_scalar(o8[:], v[:], bet[:, cs], None, add)
                else:
                    nc.vector.tensor_tensor(o8[:], v[:], u[:], op=add)
                if wide:
                    k16 = bf16_out.index(ci)
                    nc.sync.dma_start(outT16[k16 * P : (k16 + 1) * P, :], o8[:])
                else:
                    nc.sync.dma_start(outT8[ci * P : (ci + 1) * P, :], o8[:])

        for cj in range(TT // 2):
            isrc = idx0_sb if cj == 0 else idx_sb
            a_t = g_pool.tile([P, 2, batch], U8, tag="ga")
            nc.gpsimd.dma_gather(
                a_t[:], xT8[:], isrc[:, cj * 32 : cj * 32 + 16], 256, 256, batch,
                queue_num=cj % 2,
            )
            b_t = g_pool.tile([P, 2, batch], BF16, tag="gb16")
            nc.gpsimd.dma_gather(
                b_t[:], xT16[:], isrc[:, cj * 32 + 16 : cj * 32 + 32], 256, 256,
                batch, queue_num=(cj + 1) % 2,
            )
            emit_front(cj, a_t, b_t)
            if cj > 0:
                emit_back(cj - 1)
        emit_back(TT // 2 - 1)

    nc.compile()
    return nc


def _pack_idx(idx_a, idx_b):
    """Host-side int16 gather-index buffer for one core.

    Per 256-feature chunk-PAIR: 256 idx_a then 256 idx_b (one dma_gather
    each). dma_gather consumes index i from partition i%16, column i//16
    (replicated across the 8 groups of 16 partitions).
    """
    cols = []
    for f0 in range(0, len(idx_a), 2 * P):
        ids = np.concatenate(
            [idx_a[f0 : f0 + 2 * P], idx_b[f0 : f0 + 2 * P]]
        ).astype(np.int16)
        blk = ids.reshape(32, 16)  # [col, partition-within-16]
        cols.append(np.tile(blk.T, (P // 16, 1)))  # [128, 32]
    return np.ascontiguousarray(np.concatenate(cols, axis=1))


def _core_perm(weights, lo, hi):
    """Order this core's features: well-conditioned first, the 128 worst
    (by the bf16-magnitude metric of the division form) into the last
    chunk. Returns global feature indices in on-device order."""
    w = weights[lo:hi].astype(np.float64)
    e = np.exp(w - w.max(axis=1, keepdims=True))
    sm = e / e.sum(axis=1, keepdims=True)
    c = sm @ _GATE_M
    c0, c1, c2, c3 = c.T
    with np.errstate(divide="ignore", invalid="ignore"):
        al = c1 / c3
        vs = np.stack([SCALE * c2, SCALE * (c3 + c2)])  # v' at a in {0,1}
        bs = np.stack([al, 1.0 + al])  # b' at b in {0,1}
        m_max = np.max(np.abs(vs[:, None, :] * bs[None, :, :]), axis=(0, 1))
    metric = np.where(np.isfinite(m_max), m_max, np.inf)
    order = np.argsort(metric, kind="stable")
    safe = np.sort(order[: hi - lo - P])
    unsafe = np.sort(order[hi - lo - P :])
    # worst 128 features land in the unsafe chunk; safe ones fill the rest
    uci = CONFIG["unsafe_ci"]
    return lo + np.concatenate([safe[: uci * P], unsafe, safe[uci * P :]])


_NC_CACHE = {}


def _get_nc():
    key = (IN_DIM, F_CORE, BATCH, tuple(sorted(CONFIG.items())))
    if key not in _NC_CACHE:
        _NC_CACHE[key] = _build_nc(IN_DIM, F_CORE, BATCH, dict(CONFIG))
    return _NC_CACHE[key]


TRACE = False  # set by dev harness to capture an NTFF profile
LAST_RESULT = None


def kernel(x, weights, idx_a, idx_b):
    global LAST_RESULT
    import ml_dtypes
    from concourse.bass_utils import run_bass_kernel_spmd

    x = np.asarray(x, dtype=np.float32)
    weights = np.asarray(weights, dtype=np.float32)
    idx_a = np.asarray(idx_a)
    idx_b = np.asarray(idx_b)

    nc = _get_nc()
    xT8 = np.ascontiguousarray(np.rint(x * 255.0).astype(np.uint8).T)
    xT16 = np.ascontiguousarray(x.astype(ml_dtypes.bfloat16).T)
    in_maps = []
    perms = []
    for k in range(N_CORES):
        lo, hi = k * F_CORE, (k + 1) * F_CORE
        perm = _core_perm(weights, lo, hi)
        perms.append(perm)
        in_maps.append(
            {
                "xT8": xT8,
                "xT16": xT16,
                "w": np.ascontiguousarray(weights[perm]),
                "idx": _pack_idx(idx_a[perm], idx_b[perm]),
            }
        )

    res = run_bass_kernel_spmd(nc, in_maps, list(range(N_CORES)), trace=TRACE)
    LAST_RESULT = res
    bf16_out = list(CONFIG["bf16_out_chunks"])
    out = np.empty((BATCH, OUT_DIM), dtype=np.float32)
    for k in range(N_CORES):
        q = res.results[k]["outT8"].astype(np.float32)
        if bf16_out:
            q16 = res.results[k]["outT16"].astype(np.float32)
            for k16, ci in enumerate(bf16_out):
                q[ci * P : (ci + 1) * P] = q16[k16 * P : (k16 + 1) * P]
        out[:, perms[k]] = ((q - CAST_GAMMA) / SCALE).T
    return out


# revision 24
# speedup vs baseline: 1.1117x; 1.0488x over previous
"""Trainium2 Bass kernel for nn_LogicLayer (difflogic LogicLayer forward).

Computation (reference):
    w  = softmax(weights, axis=-1)            # [OUT, 16]
    c  = w @ GATE_M                           # [OUT, 4]
    a  = x[:, idx_a]; b = x[:, idx_b]         # [B, OUT] feature gathers
    out = c0 + c1*a + c2*b + c3*(a*b)

Strategy (8 NeuronCores, feature-parallel, division-form math):
  - x uploaded transposed twice: xT8 (u8, q=rint(x*255)) for a-gathers,
    xT16 (bf16) for b-gathers. Each core: 2048 features x full batch,
    16 chunks of 128 features.
  - Division form:  out = (c3*a + c2)*(b + c1/c3) + (c0 - c1*c2/c3).
    With the output code o = SCALE*out + OFF this becomes
        v' = S3*qa + S2          (ScalarE IDENT: free per-partition affine)
        b' = b + alpha           (DVE tensor_scalar add, bf16 4x mode —
                                  or ScalarE IDENT on some chunks, balance)
        m  = v' * b'             (DVE tensor_tensor, bf16 2x)
        o8 = m + beta' -> u8     (DVE tensor_scalar add, u8 out, 2x_2p)
    All three non-TT ops are per-partition-scalar affines, so the only
    expensive TT is the multiply, and the final pass emits u8 directly
    (plain HWDGE store, half the output bytes, no cast DMA).
  - Features where alpha = c1/c3 is ill-conditioned (|c3| small) are
    host-permuted into ONE "unsafe" chunk per core (chunk 15) computed
    with the classic form u + v*b (final TT at 1x, only 1 chunk pays).
    Host inverse-permutes the output columns.
  - Gate coefficients computed on-device from `weights` (exp on ScalarE,
    strided-AP reductions on VectorE), as in the reference softmax path.
"""

import numpy as np

BATCH, IN_DIM, OUT_DIM = 4096, 16384, 16384
N_CORES = 8
F_CORE = OUT_DIM // N_CORES  # 2048 output features per core
P = 128
N_CHUNKS = F_CORE // P  # 16

# Build-time tuning knobs (bake-off parameterization):
#   unsafe_ci: chunk index holding ill-conditioned features
#   b_on_act:  chunks whose b-shift runs on ScalarE (engine balance)
#   dummy_exp: issue a 1-column Exp first so the ~2.7us ACT table load
#              overlaps the input DMAs instead of the coefficient chain
CONFIG = {
    # unsafe chunk first: its DVE ops are all TTs, which do not contend
    # with the Q7s' shared SBUF port during the gather descriptor burst
    "unsafe_ci": 0,
    "b_on_act": (1, 3, 5, 9, 11, 13, 15),
    "dummy_exp": True,
    # chunks whose output is written as bf16 codes instead of u8: their
    # final TS runs at 4x (1.2us vs 2.35us) at the cost of +0.5 MB HBM
    "bf16_out_chunks": (0,),
    # gather pair 0 as four single-chunk gathers: smaller first DMAs ->
    # first v'/b' start sooner (completion sems decouple a from b)
    "split_first_pair": False,
}

SCALE = 248.0  # out -> u8 code scale
OFF = 3.5  # keeps codes in [~1.5, ~253.5]: safe from wrap/saturate
CAST_GAMMA = 3.5  # host-side un-bias (3.0 if conversion truncates)

_GATE_M = np.array(
    [
        [0.0, 0.0, 0.0, 0.0], [0.0, 0.0, 0.0, 1.0],
        [0.0, 1.0, 0.0, -1.0], [0.0, 1.0, 0.0, 0.0],
        [0.0, 0.0, 1.0, -1.0], [0.0, 0.0, 1.0, 0.0],
        [0.0, 1.0, 1.0, -2.0], [0.0, 1.0, 1.0, -1.0],
        [1.0, -1.0, -1.0, 1.0], [1.0, -1.0, -1.0, 2.0],
        [1.0, 0.0, -1.0, 0.0], [1.0, 0.0, -1.0, 1.0],
        [1.0, -1.0, 0.0, 0.0], [1.0, -1.0, 0.0, 1.0],
        [1.0, 0.0, 0.0, -1.0], [1.0, 0.0, 0.0, 0.0],
    ],
    dtype=np.float64,
)


def _build_nc(in_dim, feat_core, batch, cfg):
    """Build + compile the per-core Bass program (SPMD, identical cores)."""
    UNSAFE_CI = cfg["unsafe_ci"]
    from contextlib import ExitStack

    import concourse.bacc as bacc
    import concourse.mybir as mybir
    import concourse.tile as tile

    F32 = mybir.dt.float32
    BF16 = mybir.dt.bfloat16
    U8 = mybir.dt.uint8
    I16 = mybir.dt.int16
    TT = feat_core // P  # feature chunks per core (16)
    mult = mybir.AluOpType.mult
    add = mybir.AluOpType.add
    subtract = mybir.AluOpType.subtract
    Ident = mybir.ActivationFunctionType.Identity

    nc = bacc.Bacc(
        "TRN2", target_bir_lowering=False, debug=False, num_swdge_queues=2
    )
    xT8 = nc.dram_tensor("xT8", [in_dim, batch], U8, kind="ExternalInput")
    xT16 = nc.dram_tensor("xT16", [in_dim, batch], BF16, kind="ExternalInput")
    w = nc.dram_tensor("w", [feat_core, 16], F32, kind="ExternalInput")
    # combined gather indices: per chunk, 128 idx_a then 128 idx_b
    idx = nc.dram_tensor("idx", [P, 2 * feat_core // 16], I16, kind="ExternalInput")
    outT8 = nc.dram_tensor("outT8", [feat_core, batch], U8, kind="ExternalOutput")
    n16 = len(cfg["bf16_out_chunks"])
    outT16 = (
        nc.dram_tensor("outT16", [n16 * P, batch], BF16, kind="ExternalOutput")
        if n16
        else None
    )

    with tile.TileContext(nc) as tc, ExitStack() as ctx:
        const_pool = ctx.enter_context(tc.tile_pool(name="const", bufs=1))
        g_pool = ctx.enter_context(tc.tile_pool(name="g", bufs=3))
        uv_pool = ctx.enter_context(tc.tile_pool(name="uv", bufs=4))

        # Warmup: a tiny gather on a zeroed index tile, issued before any
        # data arrives, pays the ~6us one-time Q7 ext-isa IRAM load while
        # the idx/weights DMAs and coefficient math run.
        wu_idx = const_pool.tile([P, 1], I16, tag="wuidx")
        nc.gpsimd.memset(wu_idx[:], 0)
        wu_out = const_pool.tile([P, 1, batch], U8, tag="wuout")
        nc.gpsimd.dma_gather(
            wu_out[:], xT8[:], wu_idx[:], 16, 16, batch, queue_num=0
        )

        if cfg["dummy_exp"]:
            # prefetch the exp table set while input DMAs run
            dxp = const_pool.tile([P, 1], F32, tag="dxp")
            nc.vector.memset(dxp[:], 0.0)
            nc.scalar.activation(
                dxp[:], dxp[:], mybir.ActivationFunctionType.Exp
            )

        # pair-0 indices in their own tiny tile so the first gather only
        # waits on a 64 B/partition DMA, not the full index load
        idx0_sb = const_pool.tile([P, 32], I16, tag="idx0")
        nc.sync.dma_start(idx0_sb[:], idx[:, 0:32])
        idx_sb = const_pool.tile([P, 2 * feat_core // 16], I16, tag="idx")
        nc.sync.dma_start(idx_sb[:, 32:], idx[:, 32:])

        sc2 = const_pool.tile([P, TT], F32, tag="sc2")  # SCALE*c2
        sc3 = const_pool.tile([P, TT], F32, tag="sc3")  # SCALE*c3/255
        alp = const_pool.tile([P, TT], F32, tag="alp")  # c1/c3
        bet = const_pool.tile([P, TT], F32, tag="bet")  # SCALE*(c0-c1c2/c3)+OFF
        uc0 = const_pool.tile([P, TT], F32, tag="uc0")  # SCALE*c0 + OFF
        uc1 = const_pool.tile([P, TT], F32, tag="uc1")  # SCALE*c1/255

        # ---------- gate coefficients ----------
        # Setup pool stays open for the kernel's lifetime (~5 KB/partition):
        # closing it would put a scope-exit barrier in front of the first
        # gather (~8 us of serialized lead-in).
        sp = ctx.enter_context(tc.tile_pool(name="setup", bufs=1))
        if True:
            w_sb = sp.tile([P, TT, 16], F32, tag="wsb")
            nc.sync.dma_start(w_sb[:], w[:].rearrange("(t p) g -> p t g", p=P))
            E = sp.tile([P, TT, 16], F32, tag="E")
            nc.scalar.activation(E[:], w_sb[:], mybir.ActivationFunctionType.Exp)

            su = sp.tile([P, TT], F32, tag="su")
            nc.vector.reduce_sum(su[:], E[:], axis=mybir.AxisListType.X)
            r = sp.tile([P, TT], F32, tag="r")
            nc.vector.reciprocal(r[:], su[:])

            c0u = sp.tile([P, TT], F32, tag="c0u")
            nc.vector.reduce_sum(c0u[:], E[:, :, 8:16], axis=mybir.AxisListType.X)

            E4 = E[:].rearrange("p t (g2 g1) -> p t g2 g1", g1=4)
            a1 = sp.tile([P, TT], F32, tag="a1")
            nc.vector.reduce_sum(a1[:], E4[:, :, 0:2, 2:4], axis=mybir.AxisListType.XY)
            b1 = sp.tile([P, TT], F32, tag="b1")
            nc.vector.reduce_sum(b1[:], E4[:, :, 2:4, 0:2], axis=mybir.AxisListType.XY)
            c1u = sp.tile([P, TT], F32, tag="c1u")
            nc.vector.tensor_tensor(c1u[:], a1[:], b1[:], op=subtract)

            a2 = sp.tile([P, TT], F32, tag="a2")
            nc.vector.reduce_sum(a2[:], E[:, :, 4:8], axis=mybir.AxisListType.X)
            b2 = sp.tile([P, TT], F32, tag="b2")
            nc.vector.reduce_sum(b2[:], E[:, :, 8:12], axis=mybir.AxisListType.X)
            c2u = sp.tile([P, TT], F32, tag="c2u")
            nc.vector.tensor_tensor(c2u[:], a2[:], b2[:], op=subtract)

            # c3 = (E1+E8) + (E11+E13) - (E2+E4) - (E7+E14) - 2*(E6-E9)
            def eg(g):
                return E[:, :, g : g + 1]

            p1 = sp.tile([P, TT, 1], F32, tag="p1")
            nc.vector.tensor_tensor(p1[:], eg(1), eg(8), op=add)
            p2 = sp.tile([P, TT, 1], F32, tag="p2")
            nc.vector.tensor_tensor(p2[:], eg(11), eg(13), op=add)
            n1 = sp.tile([P, TT, 1], F32, tag="n1")
            nc.vector.tensor_tensor(n1[:], eg(2), eg(4), op=add)
            n2 = sp.tile([P, TT, 1], F32, tag="n2")
            nc.vector.tensor_tensor(n2[:], eg(7), eg(14), op=add)
            d6 = sp.tile([P, TT, 1], F32, tag="d6")
            nc.vector.tensor_tensor(d6[:], eg(6), eg(9), op=subtract)
            pp = sp.tile([P, TT, 1], F32, tag="pp")
            nc.vector.tensor_tensor(pp[:], p1[:], p2[:], op=add)
            nn_ = sp.tile([P, TT, 1], F32, tag="nn")
            nc.vector.tensor_tensor(nn_[:], n1[:], n2[:], op=add)
            c3a = sp.tile([P, TT, 1], F32, tag="c3a")
            nc.vector.tensor_tensor(c3a[:], pp[:], nn_[:], op=subtract)
            c3u = sp.tile([P, TT, 1], F32, tag="c3u")
            nc.vector.scalar_tensor_tensor(
                c3u[:], d6[:], -2.0, c3a[:], op0=mult, op1=add
            )
            c3f = c3u[:, :, 0]

            # Folded coefficients. r = 1/sum(E) (softmax norm), qa = 255*a.
            #   v' = sc3*qa + sc2 = SCALE*(c3*a + c2)
            #   o  = v'*(b + alp) + bet ; alp = c1/c3 (r cancels)
            #   bet = SCALE*r*(c0u - alp*c2u) + OFF
            rS = sp.tile([P, TT], F32, tag="rS")
            nc.vector.tensor_scalar_mul(rS[:], r[:], SCALE)
            rS255 = sp.tile([P, TT], F32, tag="rS255")
            nc.vector.tensor_scalar_mul(rS255[:], r[:], SCALE / 255.0)
            nc.vector.tensor_tensor(sc2[:], c2u[:], rS[:], op=mult)
            nc.vector.tensor_tensor(sc3[:], c3f, rS255[:], op=mult)

            rc3 = sp.tile([P, TT], F32, tag="rc3")
            nc.vector.reciprocal(rc3[:], c3f)
            nc.vector.tensor_tensor(alp[:], c1u[:], rc3[:], op=mult)
            t1 = sp.tile([P, TT], F32, tag="t1")
            nc.vector.tensor_tensor(t1[:], alp[:], c2u[:], op=mult)
            t2 = sp.tile([P, TT], F32, tag="t2")
            nc.vector.tensor_tensor(t2[:], c0u[:], t1[:], op=subtract)
            t3 = sp.tile([P, TT], F32, tag="t3")
            nc.vector.tensor_tensor(t3[:], t2[:], rS[:], op=mult)
            nc.vector.tensor_scalar_add(bet[:], t3[:], OFF)

            # classic-form coefficients for the unsafe chunk
            uc0a = sp.tile([P, TT], F32, tag="uc0a")
            nc.vector.tensor_tensor(uc0a[:], c0u[:], rS[:], op=mult)
            nc.vector.tensor_scalar_add(uc0[:], uc0a[:], OFF)
            nc.vector.tensor_tensor(uc1[:], c1u[:], rS255[:], op=mult)

        # ---------- main gather + FMA loop ----------
        # Gathers are issued per chunk-PAIR (256 indices each) to halve the
        # SWDGE descriptor-gen op count on the Q7s (which 2-port DVE ops
        # lock out of SBUF).
        #
        # Emission order is software-pipelined: engine queues are strict
        # FIFO, so per pair we emit gathers, then BOTH v' passes (a-fed),
        # then both b' passes (b-fed), and the DVE finals (m, o8) of the
        # PREVIOUS pair. This keeps every queue's head runnable — no pass
        # sits behind one that waits on a later DMA or another engine.
        o_pool = ctx.enter_context(tc.tile_pool(name="o", bufs=4))
        B_ON_ACT = set(cfg["b_on_act"])  # b-shift on ScalarE (balance)

        state = {}  # ci -> (v_tile, b_operand, u_tile or None)

        def emit_front(cj, a_views, b_views):
            for sub in range(2):
                ci = 2 * cj + sub
                cs = slice(ci, ci + 1)
                a_v = a_views[sub]
                # v' = sc3*qa + sc2  (ScalarE, free per-partition affine)
                v = uv_pool.tile([P, batch], BF16, tag="v")
                nc.scalar.activation(
                    v[:], a_v, Ident, bias=sc2[:, cs], scale=sc3[:, cs]
                )
                state[ci] = [v, None, None]
                if ci == UNSAFE_CI:
                    u = uv_pool.tile([P, batch], BF16, tag="bp")
                    nc.scalar.activation(
                        u[:], a_v, Ident, bias=uc0[:, cs], scale=uc1[:, cs]
                    )
                    state[ci][1] = b_views[sub]
                    state[ci][2] = u
            for sub in range(2):
                ci = 2 * cj + sub
                cs = slice(ci, ci + 1)
                if ci == UNSAFE_CI:
                    continue
                b_v = b_views[sub]
                bp = uv_pool.tile([P, batch], BF16, tag="bp")
                if ci in B_ON_ACT:
                    nc.scalar.activation(bp[:], b_v, Ident, bias=alp[:, cs])
                else:
                    nc.vector.tensor_scalar(bp[:], b_v, alp[:, cs], None, add)
                state[ci][1] = bp[:]

        bf16_out = list(cfg["bf16_out_chunks"])

        def emit_back(cj):
            for sub in range(2):
                ci = 2 * cj + sub
                cs = slice(ci, ci + 1)
                v, b_op, u = state.pop(ci)
                wide = ci in bf16_out
                o8 = o_pool.tile([P, batch], BF16 if wide else U8,
                                 tag="o16" if wide else "o8")
                nc.vector.tensor_tensor(v[:], v[:], b_op, op=mult)
                if u is None:
                    nc.vector.tensor_scalar(o8[:], v[:], bet[:, cs], None, add)
                else:
                    nc.vector.tensor_tensor(o8[:], v[:], u[:], op=add)
                if wide:
                    k16 = bf16_out.index(ci)
                    nc.sync.dma_start(outT16[k16 * P : (k16 + 1) * P, :], o8[:])
                else:
                    nc.sync.dma_start(outT8[ci * P : (ci + 1) * P, :], o8[:])

        for cj in range(TT // 2):
            isrc = idx0_sb if cj == 0 else idx_sb
            if cj == 0 and cfg["split_first_pair"]:
                av, bv = [], []
                for sub in range(2):
                    at = const_pool.tile([P, 1, batch], U8, tag=f"ga{sub}")
                    nc.gpsimd.dma_gather(
                        at[:], xT8[:], isrc[:, sub * 8 : sub * 8 + 8],
                        128, 128, batch, queue_num=sub % 2,
                    )
                    av.append(at[:, 0, :])
                for sub in range(2):
                    bt = const_pool.tile([P, 1, batch], BF16, tag=f"gb{sub}")
                    nc.gpsimd.dma_gather(
                        bt[:], xT16[:], isrc[:, 16 + sub * 8 : 16 + sub * 8 + 8],
                        128, 128, batch, queue_num=sub % 2,
                    )
                    bv.append(bt[:, 0, :])
                emit_front(0, av, bv)
                continue
            a_t = g_pool.tile([P, 2, batch], U8, tag="ga")
            nc.gpsimd.dma_gather(
                a_t[:], xT8[:], isrc[:, cj * 32 : cj * 32 + 16], 256, 256, batch,
                queue_num=cj % 2,
            )
            b_t = g_pool.tile([P, 2, batch], BF16, tag="gb16")
            nc.gpsimd.dma_gather(
                b_t[:], xT16[:], isrc[:, cj * 32 + 16 : cj * 32 + 32], 256, 256,
                batch, queue_num=(cj + 1) % 2,
            )
            emit_front(cj, [a_t[:, 0, :], a_t[:, 1, :]], [b_t[:, 0, :], b_t[:, 1, :]])
            if cj > 0:
                emit_back(cj - 1)
        emit_back(TT // 2 - 1)

    nc.compile()
    return nc


def _pack_idx(idx_a, idx_b):
    """Host-side int16 gather-index buffer for one core.

    Per 256-feature chunk-PAIR: 256 idx_a then 256 idx_b (one dma_gather
    each). dma_gather consumes index i from partition i%16, column i//16
    (replicated across the 8 groups of 16 partitions).
    """
    cols = []
    for f0 in range(0, len(idx_a), 2 * P):
        ids = np.concatenate(
            [idx_a[f0 : f0 + 2 * P], idx_b[f0 : f0 + 2 * P]]
        ).astype(np.int16)
        blk = ids.reshape(32, 16)  # [col, partition-within-16]
        cols.append(np.tile(blk.T, (P // 16, 1)))  # [128, 32]
    return np.ascontiguousarray(np.concatenate(cols, axis=1))


def _core_perm(weights, lo, hi):
    """Order this core's features: well-conditioned first, the 128 worst
    (by the bf16-magnitude metric of the division form) into the last
    chunk. Returns global feature indices in on-device order."""
    w = weights[lo:hi].astype(np.float64)
    e = np.exp(w - w.max(axis=1, keepdims=True))
    sm = e / e.sum(axis=1, keepdims=True)
    c = sm @ _GATE_M
    c0, c1, c2, c3 = c.T
    with np.errstate(divide="ignore", invalid="ignore"):
        al = c1 / c3
        vs = np.stack([SCALE * c2, SCALE * (c3 + c2)])  # v' at a in {0,1}
        bs = np.stack([al, 1.0 + al])  # b' at b in {0,1}
        m_max = np.max(np.abs(vs[:, None, :] * bs[None, :, :]), axis=(0, 1))
    metric = np.where(np.isfinite(m_max), m_max, np.inf)
    order = np.argsort(metric, kind="stable")
    safe = np.sort(order[: hi - lo - P])
    unsafe = np.sort(order[hi - lo - P :])
    # worst 128 features land in the unsafe chunk; safe ones fill the rest
    uci = CONFIG["unsafe_ci"]
    return lo + np.concatenate([safe[: uci * P], unsafe, safe[uci * P :]])


_NC_CACHE = {}


def _get_nc():
    key = (IN_DIM, F_CORE, BATCH, tuple(sorted(CONFIG.items())))
    if key not in _NC_CACHE:
        _NC_CACHE[key] = _build_nc(IN_DIM, F_CORE, BATCH, dict(CONFIG))
    return _NC_CACHE[key]


TRACE = False  # set by dev harness to capture an NTFF profile
LAST_RESULT = None


def kernel(x, weights, idx_a, idx_b):
    global LAST_RESULT
    import ml_dtypes
    from concourse.bass_utils import run_bass_kernel_spmd

    x = np.asarray(x, dtype=np.float32)
    weights = np.asarray(weights, dtype=np.float32)
    idx_a = np.asarray(idx_a)
    idx_b = np.asarray(idx_b)

    nc = _get_nc()
    xT8 = np.ascontiguousarray(np.rint(x * 255.0).astype(np.uint8).T)
    xT16 = np.ascontiguousarray(x.astype(ml_dtypes.bfloat16).T)
    in_maps = []
    perms = []
    for k in range(N_CORES):
        lo, hi = k * F_CORE, (k + 1) * F_CORE
        perm = _core_perm(weights, lo, hi)
        perms.append(perm)
        in_maps.append(
            {
                "xT8": xT8,
                "xT16": xT16,
                "w": np.ascontiguousarray(weights[perm]),
                "idx": _pack_idx(idx_a[perm], idx_b[perm]),
            }
        )

    res = run_bass_kernel_spmd(nc, in_maps, list(range(N_CORES)), trace=TRACE)
    LAST_RESULT = res
    bf16_out = list(CONFIG["bf16_out_chunks"])
    out = np.empty((BATCH, OUT_DIM), dtype=np.float32)
    for k in range(N_CORES):
        q = res.results[k]["outT8"].astype(np.float32)
        if bf16_out:
            q16 = res.results[k]["outT16"].astype(np.float32)
            for k16, ci in enumerate(bf16_out):
                q[ci * P : (ci + 1) * P] = q16[k16 * P : (k16 + 1) * P]
        out[:, perms[k]] = ((q - CAST_GAMMA) / SCALE).T
    return out
